# revision 16
# baseline (speedup 1.0000x reference)
"""Multi-head attention (B=2, N=2048, D=1024, H=16) on 8 Trainium2 cores.

Sharding: data-parallel over batch (cores 0-3 -> b=0, cores 4-7 -> b=1) and
tensor-parallel over heads (4 heads per core, i.e. 256 of the 1024 QKV/O
channels).  Each core computes its 4 heads' attention plus a partial output
projection; the host sums the 4 partials per batch and adds bo.

Projections and output projection run in float32r (fp32 data, full-rate PE
mode).  The attention matmuls (QK^T scores and PV) run in bf16 operands with
fp32 PSUM accumulation: f32r matmuls self-load their 4-byte stationary
operand (~430ns serial per matmul), which starves the PE; bf16 weight loads
hide completely.
"""

import numpy as np

import concourse.bass as bass
import concourse.bacc as bacc
import concourse.tile as tile
from concourse import mybir
from concourse.bass_utils import run_bass_kernel_spmd

F32 = mybir.dt.float32
F32R = mybir.dt.float32r
BF16 = mybir.dt.bfloat16
AF = mybir.ActivationFunctionType

B, N, D, H, HD = 2, 2048, 1024, 16, 64
E = 256            # channels per core (4 heads * 64)
DC = D // 128      # 8 contraction chunks for projections
NB = N // 128      # 16 token blocks / k chunks
SCALE = 1.0 / np.sqrt(HD)
DT_PR = BF16       # dtype for projection matmul operands (x, Wq/Wk/Wv)
DT_SC = BF16       # dtype for scores matmul operands (qt/kt)
DT_PV = BF16       # dtype for PV matmul operands (vp, w=exp out)
DT_AT = BF16       # dtype for output-projection operands (attnT, WoT)


def _emit(nc):
    xT = nc.dram_tensor("xT", [D, N], DT_PR, kind="ExternalInput")
    wqT = nc.dram_tensor("wqT", [D, E], DT_PR, kind="ExternalInput")
    wkT = nc.dram_tensor("wkT", [D, E], DT_PR, kind="ExternalInput")
    wvT = nc.dram_tensor("wvT", [D, E], DT_PR, kind="ExternalInput")
    woT = nc.dram_tensor("woT", [E, D], DT_AT, kind="ExternalInput")
    bq2 = nc.dram_tensor("bq2", [128, 2], F32, kind="ExternalInput")
    bk2 = nc.dram_tensor("bk2", [128, 2], F32, kind="ExternalInput")
    bv1 = nc.dram_tensor("bv1", [E], F32, kind="ExternalInput")
    vones = nc.dram_tensor("vones", [128, NB, 4], DT_PV, kind="ExternalInput")
    onesr = nc.dram_tensor("onesr", [1, HD], F32R, kind="ExternalInput")
    out = nc.dram_tensor("out", [N, D], F32, kind="ExternalOutput")

    with tile.TileContext(nc) as tc:
        with tc.tile_pool(name="per", bufs=1) as per, \
             tc.tile_pool(name="wp", bufs=8) as wp, \
             tc.tile_pool(name="dn", bufs=2) as dn, \
             tc.tile_pool(name="up", bufs=2) as up, \
             tc.tile_pool(name="op", bufs=2) as op, \
             tc.tile_pool(name="ps", bufs=1, space="PSUM") as ps:

            # ---- persistent SBUF tiles ----
            xt = per.tile([128, DC, N], DT_PR)          # x[b].T  (d-chunk, tokens)
            wq = per.tile([128, DC, E], DT_PR)
            wk = per.tile([128, DC, E], DT_PR)
            wv = per.tile([128, DC, E], DT_PR)
            wo = per.tile([128, 2, D], DT_AT)          # WoT (e-chunk)
            qt = per.tile([128, 2, N], DT_SC)          # Q^T packed: pair, head-half
            kt = per.tile([128, 2, N], DT_SC)
            vp = per.tile([128, NB, 4, HD + 4], DT_PV)  # V + ones col, padded stride
            at = per.tile([128, 2, N], DT_AT)          # attn^T normalized
            bqs = per.tile([128, 2], F32)
            bks = per.tile([128, 2], F32)
            bvb = per.tile([128, E], F32)              # bv broadcast across parts
            ones = per.tile([1, HD], F32R)

            for dc in range(DC):
                nc.sync.dma_start(out=xt[:, dc, :], in_=xT[dc * 128:(dc + 1) * 128, :])
                nc.sync.dma_start(out=wq[:, dc, :], in_=wqT[dc * 128:(dc + 1) * 128, :])
                nc.sync.dma_start(out=wk[:, dc, :], in_=wkT[dc * 128:(dc + 1) * 128, :])
                nc.sync.dma_start(out=wv[:, dc, :], in_=wvT[dc * 128:(dc + 1) * 128, :])
            for ec in range(2):
                nc.sync.dma_start(out=wo[:, ec, :], in_=woT[ec * 128:(ec + 1) * 128, :])
            nc.sync.dma_start(out=bqs, in_=bq2[:, :])
            nc.sync.dma_start(out=bks, in_=bk2[:, :])
            bv_ap = bv1[:]
            nc.gpsimd.dma_start(
                out=bvb,
                in_=bass.AP(tensor=bv_ap.tensor, offset=0, ap=[[0, 128], [1, E]]),
            )
            nc.sync.dma_start(out=ones, in_=onesr[:, :])
            nc.sync.dma_start(out=vp[:, :, :, HD:HD + 1],
                              in_=vones[:, :, :].rearrange("p a (b o) -> p a b o", o=1))

            pj_n = [0]

            def pj_tag():
                pj_n[0] += 1
                return ("pjA", "pjB")[pj_n[0] % 2]

            # ---- projections (1-bank pj psum tiles; overlap with attention) ----
            def qk_proj(pair):
                for wsb, dst, bias in ((wk, kt, bks), (wq, qt, bqs)):
                    for n4 in range(4):
                        pt = ps.tile([128, 512], F32, tag=pj_tag(), name=f"pp{pair}{n4}")
                        for dc in range(DC):
                            nc.tensor.matmul(
                                pt[:, :],
                                wsb[:, dc, pair * 128:(pair + 1) * 128],
                                xt[:, dc, n4 * 512:(n4 + 1) * 512],
                                start=(dc == 0), stop=(dc == DC - 1),
                            )
                        nc.vector.tensor_scalar_add(
                            dst[:, pair, n4 * 512:(n4 + 1) * 512], pt[:, :],
                            bias[:, pair:pair + 1],
                        )

            def v_proj():
                for nb in range(NB):
                    pt = ps.tile([128, E], F32, tag=pj_tag(), name=f"pv{nb}")
                    for dc in range(DC):
                        nc.tensor.matmul(
                            pt[:, :],
                            xt[:, dc, nb * 128:(nb + 1) * 128],
                            wv[:, dc, :],
                            start=(dc == 0), stop=(dc == DC - 1),
                        )
                    nc.vector.tensor_add(
                        vp[:, nb, :, 0:HD],
                        pt.rearrange("p (h d) -> p h d", h=4),
                        bvb.rearrange("p (h d) -> p h d", h=4),
                    )

            # ---- attention for one head: k-loop with s1/s2 double buffering ----
            def attn_head(pair, q2, hh):
                q0 = q2 * 1024
                p0 = hh * 64
                pv = ps.tile([HD + 1, 1024], F32, tag="pv", name=f"pv{pair}{q2}{hh}")
                for k in range(NB):
                    st = ps.tile([128, 1024], F32, tag=("s1", "s2")[k % 2],
                                 name=f"st{k}")
                    for half in range(2):
                        nc.tensor.matmul(
                            st[:, half * 512:(half + 1) * 512],
                            kt[p0:p0 + 64, pair, k * 128:(k + 1) * 128],
                            qt[p0:p0 + 64, pair,
                               q0 + half * 512:q0 + (half + 1) * 512],
                            start=True, stop=True,
                            tile_position=(p0, 0),
                        )
                    w = wp.tile([128, 1024], DT_PV, tag="w", name=f"w{k}")
                    nc.scalar.activation(w, st, AF.Exp, scale=SCALE)
                    for half in range(2):
                        nc.tensor.matmul(
                            pv[:, half * 512:(half + 1) * 512],
                            vp[:, k, 2 * pair + hh, 0:HD + 1],
                            w[:, half * 512:(half + 1) * 512],
                            start=(k == 0), stop=(k == NB - 1),
                        )
                # normalize: attn^T[d, q] / den[q]
                den = dn.tile([1, 1024], F32, tag="den", name="den")
                rec = dn.tile([1, 1024], F32, tag="rec", name="rec")
                recr = dn.tile([1, 1024], F32R, tag="recr", name="recr")
                u = up.tile([HD, 1024], F32)
                nc.vector.tensor_copy(den, pv[HD:HD + 1, :])
                nc.vector.tensor_copy(u, pv[0:HD, :])
                nc.vector.reciprocal_approx_fast(rec, den)
                nc.vector.tensor_copy(recr, rec)
                bc = ps.tile([HD, 1024], F32, tag=("s1", "s2")[NB % 2], name="bc")
                for half in range(2):
                    nc.tensor.matmul(
                        bc[:, half * 512:(half + 1) * 512],
                        ones[:, :],
                        recr[:, half * 512:(half + 1) * 512],
                        start=True, stop=True,
                    )
                nc.vector.tensor_mul(
                    at[p0:p0 + 64, pair, q0:q0 + 1024], u, bc)

            # ---- output projection for one q-half ----
            def oproj(q2):
                for nb in range(q2 * 8, q2 * 8 + 8):
                    ot = op.tile([128, 1024], F32)
                    for half in range(2):
                        po = ps.tile([128, 512], F32, tag=pj_tag(), name=f"po{nb}")
                        for ec in range(2):
                            nc.tensor.matmul(
                                po[:, :],
                                at[:, ec, nb * 128:(nb + 1) * 128],
                                wo[:, ec, half * 512:(half + 1) * 512],
                                start=(ec == 0), stop=(ec == 1),
                            )
                        nc.vector.tensor_copy(ot[:, half * 512:(half + 1) * 512], po)
                    nc.sync.dma_start(out=out[nb * 128:(nb + 1) * 128, :], in_=ot)

            qk_proj(0)
            v_proj()
            qk_proj(1)
            for q2 in range(2):
                for pair in range(2):
                    for hh in range(2):
                        attn_head(pair, q2, hh)
                oproj(q2)
    return nc


_CACHE = {}


def _build():
    if "nc" not in _CACHE:
        nc = bacc.Bacc("TRN2", target_bir_lowering=False, debug=False)
        _emit(nc)
        nc.compile()
        _CACHE["nc"] = nc
    return _CACHE["nc"]


def make_in_maps(x, Wq, bq, Wk, bk, Wv, bv, Wo, bo):
    import ml_dtypes
    f32 = np.float32
    bt = ml_dtypes.bfloat16
    dpr = bt if DT_PR == BF16 else f32
    ones_np = np.ones((128, NB, 4), bt if DT_PV == BF16 else f32)
    xTs = [np.ascontiguousarray(np.asarray(x[b], dtype=f32).T).astype(dpr)
           for b in range(B)]
    in_maps = []
    for c in range(8):
        b, r0 = c // 4, (c % 4) * E
        rows = slice(r0, r0 + E)
        in_maps.append({
            "xT": xTs[b],
            "wqT": np.ascontiguousarray(np.asarray(Wq, f32)[rows].T).astype(dpr),
            "wkT": np.ascontiguousarray(np.asarray(Wk, f32)[rows].T).astype(dpr),
            "wvT": np.ascontiguousarray(np.asarray(Wv, f32)[rows].T).astype(dpr),
            "woT": np.ascontiguousarray(np.asarray(Wo, f32)[:, rows].T).astype(bt if DT_AT == BF16 else f32),
            "bq2": np.ascontiguousarray(np.asarray(bq, f32)[rows].reshape(2, 128).T),
            "bk2": np.ascontiguousarray(np.asarray(bk, f32)[rows].reshape(2, 128).T),
            "bv1": np.ascontiguousarray(np.asarray(bv, f32)[rows]),
            "vones": ones_np,
            "onesr": np.ones((1, HD), f32),
        })
    return in_maps


def kernel(x, Wq, bq, Wk, bk, Wv, bv, Wo, bo, _spmd_kwargs=None):
    nc = _build()
    in_maps = make_in_maps(x, Wq, bq, Wk, bk, Wv, bv, Wo, bo)
    res = run_bass_kernel_spmd(nc, in_maps, core_ids=list(range(8)),
                               **(_spmd_kwargs or {}))
    parts = np.stack([res.results[c]["out"] for c in range(8)])
    outv = parts.reshape(B, 4, N, D).sum(axis=1) + np.asarray(bo, np.float32)
    if _spmd_kwargs:
        _CACHE["last_results"] = res
    return outv.astype(np.float32)


# revision 26
# speedup vs baseline: 1.5189x; 1.5189x over previous
"""Multi-head attention (B=2, N=2048, D=1024, H=16) on 8 Trainium2 cores.

Sharding: data-parallel over batch (cores 0-3 -> b=0, cores 4-7 -> b=1) and
tensor-parallel over heads (4 heads per core = 256 of 1024 QKV/O channels).
Each core computes its 4 heads' attention plus a partial output projection;
the host sums the 4 partials per batch and adds bo.

Pipeline design (per core):
 - All projections produce transposed layouts directly (x is pre-transposed
   on the host); V^T is turned into V-natural via DMA-xbar transposes.
 - Attention runs one head at a time: scores S^T[k,q] (bf16, K=64),
   exp(scale*s) fused on ScalarE reading PSUM (FD=1024), PV accumulates
   attn^T plus a softmax-denominator row (ones column appended to V).
 - Normalization: DVE reciprocal + GPSIMD partition_broadcast + DVE multiply.
 - PSUM banks: s1(2) s2(2) pv(2) pjA(1) pjB(1); projections/O-proj run on the
   pj tags so they overlap the ScalarE-bound attention phase as fillers.
"""

import numpy as np

import concourse.bass as bass
import concourse.bacc as bacc
import concourse.tile as tile
from concourse import mybir
from concourse.bass_utils import run_bass_kernel_spmd

F32 = mybir.dt.float32
F32R = mybir.dt.float32r
BF16 = mybir.dt.bfloat16
AF = mybir.ActivationFunctionType

B, N, D, H, HD = 2, 2048, 1024, 16, 64
E = 256            # channels per core (4 heads * 64)
DC = D // 128      # 8 contraction chunks for projections
NB = N // 128      # 16 token blocks / k chunks
SCALE = 1.0 / np.sqrt(HD)
DT_PR = BF16       # projection matmul operands (x, Wq/Wk/Wv)
DT_SC = BF16       # scores matmul operands (qt/kt)
DT_PV = BF16       # PV matmul operands (vp, w=exp out)
DT_AT = BF16       # output-projection operands (attnT, WoT)


def _emit(nc):
    xT = nc.dram_tensor("xT", [D, N], DT_PR, kind="ExternalInput")
    wqT = nc.dram_tensor("wqT", [D, E], DT_PR, kind="ExternalInput")
    wkT = nc.dram_tensor("wkT", [D, E], DT_PR, kind="ExternalInput")
    wvT = nc.dram_tensor("wvT", [D, E], DT_PR, kind="ExternalInput")
    woT = nc.dram_tensor("woT", [E, D], DT_AT, kind="ExternalInput")
    bq2 = nc.dram_tensor("bq2", [128, 2], F32, kind="ExternalInput")
    bk2 = nc.dram_tensor("bk2", [128, 2], F32, kind="ExternalInput")
    bv1 = nc.dram_tensor("bv1", [E], F32, kind="ExternalInput")
    vones = nc.dram_tensor("vones", [128, NB, 4], DT_PV, kind="ExternalInput")
    out = nc.dram_tensor("out", [N, D], F32, kind="ExternalOutput")

    with tile.TileContext(nc) as tc:
        with tc.tile_pool(name="per", bufs=1) as per, \
             tc.tile_pool(name="wp", bufs=8) as wp, \
             tc.tile_pool(name="dn", bufs=2) as dn, \
             tc.tile_pool(name="up", bufs=2) as up, \
             tc.tile_pool(name="op", bufs=2) as op, \
             tc.tile_pool(name="ps", bufs=1, space="PSUM") as ps:

            # ---- persistent SBUF tiles ----
            xt = per.tile([128, DC, N], DT_PR)           # x[b].T (d-chunk, tokens)
            wq = per.tile([128, DC, E], DT_PR)
            wk = per.tile([128, DC, E], DT_PR)
            wv = per.tile([128, DC, E], DT_PR)
            wo = per.tile([128, 2, D], DT_AT)            # WoT (e-chunk)
            qt = per.tile([128, 2, N], DT_SC)            # Q^T: (pair, head-half)
            kt = per.tile([128, 2, N], DT_SC)
            vp = per.tile([128, NB, 4, 128], DT_PV)      # V natural + ones col (256B-aligned head stride for xbar transpose)
            at = per.tile([128, 2, N], DT_AT)            # attn^T normalized
            bqs = per.tile([128, 2], F32)
            bks = per.tile([128, 2], F32)
            bvb = per.tile([128, E], F32)

            for dc in range(DC):
                nc.sync.dma_start(out=xt[:, dc, :], in_=xT[dc * 128:(dc + 1) * 128, :])
                nc.sync.dma_start(out=wq[:, dc, :], in_=wqT[dc * 128:(dc + 1) * 128, :])
                nc.sync.dma_start(out=wk[:, dc, :], in_=wkT[dc * 128:(dc + 1) * 128, :])
                nc.sync.dma_start(out=wv[:, dc, :], in_=wvT[dc * 128:(dc + 1) * 128, :])
            for ec in range(2):
                nc.sync.dma_start(out=wo[:, ec, :], in_=woT[ec * 128:(ec + 1) * 128, :])
            nc.sync.dma_start(out=bqs, in_=bq2[:, :])
            nc.sync.dma_start(out=bks, in_=bk2[:, :])
            bv_ap = bv1[:]
            nc.gpsimd.dma_start(
                out=bvb,
                in_=bass.AP(tensor=bv_ap.tensor, offset=0, ap=[[0, 128], [1, E]]),
            )
            nc.sync.dma_start(out=vp[:, :, :, HD:HD + 1],
                              in_=vones[:, :, :].rearrange("p a (b o) -> p a b o", o=1))

            pj_n = [0]

            def pj_tag():
                pj_n[0] += 1
                return ("pjA", "pjB")[pj_n[0] % 2]

            # ---- filler units (each: one 1-bank psum group on a pj tag) ----
            def proj_group(wsb, dst, bias, pair, n4):
                def emit():
                    pt = ps.tile([128, 512], F32, tag=pj_tag(), name="ppj")
                    for dc in range(DC):
                        nc.tensor.matmul(
                            pt[:, :],
                            wsb[:, dc, pair * 128:(pair + 1) * 128],
                            xt[:, dc, n4 * 512:(n4 + 1) * 512],
                            start=(dc == 0), stop=(dc == DC - 1),
                        )
                    nc.vector.tensor_scalar_add(
                        dst[:, pair, n4 * 512:(n4 + 1) * 512], pt[:, :],
                        bias[:, pair:pair + 1],
                    )
                return emit

            def vnat_group(nb):
                def emit():
                    pt = ps.tile([128, E], F32, tag=pj_tag(), name="pvn")
                    for dc in range(DC):
                        nc.tensor.matmul(
                            pt[:, :],
                            xt[:, dc, nb * 128:(nb + 1) * 128],
                            wv[:, dc, :],
                            start=(dc == 0), stop=(dc == DC - 1),
                        )
                    nc.vector.tensor_add(
                        vp[:, nb, :, 0:HD],
                        pt.rearrange("p (h d) -> p h d", h=4),
                        bvb.rearrange("p (h d) -> p h d", h=4),
                    )
                return emit

            def oproj_unit(nb, half):
                def emit():
                    po = ps.tile([128, 512], F32, tag=pj_tag(), name="po")
                    for ec in range(2):
                        nc.tensor.matmul(
                            po[:, :],
                            at[:, ec, nb * 128:(nb + 1) * 128],
                            wo[:, ec, half * 512:(half + 1) * 512],
                            start=(ec == 0), stop=(ec == 1),
                        )
                    ot = op.tile([128, 512], F32, tag=f"ot{nb % 2}", name="ot")
                    nc.vector.tensor_copy(ot, po)
                    nc.sync.dma_start(
                        out=out[nb * 128:(nb + 1) * 128,
                                half * 512:(half + 1) * 512],
                        in_=ot)
                return emit

            # ---- attention for one head (16 k-iters, s1/s2 double buffer) ----
            def attn_head(pair, q2, hh, fillers):
                q0 = q2 * 1024
                p0 = hh * 64
                fi = 0
                pv = ps.tile([HD + 1, 1024], F32, tag="pv", name="pv")
                wtiles = {}
                for k in range(NB):
                    st = ps.tile([128, 1024], F32, tag=("s1", "s2")[k % 2],
                                 name="st")
                    for half in range(2):
                        nc.tensor.matmul(
                            st[:, half * 512:(half + 1) * 512],
                            kt[p0:p0 + 64, pair, k * 128:(k + 1) * 128],
                            qt[p0:p0 + 64, pair,
                               q0 + half * 512:q0 + (half + 1) * 512],
                            start=True, stop=True,
                            tile_position=(p0, 0),
                        )
                    w = wp.tile([128, 1024], DT_PV, tag="w", name="w")
                    nc.scalar.activation(w, st, AF.Exp, scale=SCALE)
                    wtiles[k] = w
                    while fi < len(fillers) and fi <= k * len(fillers) // NB:
                        fillers[fi]()
                        fi += 1
                    if k > 0:
                        wprev = wtiles.pop(k - 1)
                        for half in range(2):
                            nc.tensor.matmul(
                                pv[:, half * 512:(half + 1) * 512],
                                vp[:, k - 1, 2 * pair + hh, 0:HD + 1],
                                wprev[:, half * 512:(half + 1) * 512],
                                start=(k - 1 == 0), stop=False,
                            )
                wlast = wtiles.pop(NB - 1)
                for half in range(2):
                    nc.tensor.matmul(
                        pv[:, half * 512:(half + 1) * 512],
                        vp[:, NB - 1, 2 * pair + hh, 0:HD + 1],
                        wlast[:, half * 512:(half + 1) * 512],
                        start=False, stop=True,
                    )
                while fi < len(fillers):
                    fillers[fi]()
                    fi += 1
                # normalize: attn^T[d, q] / den[q]
                den = dn.tile([1, 1024], F32, tag="den", name="den")
                rec = dn.tile([1, 1024], F32, tag="rec", name="rec")
                bcr = up.tile([HD, 1024], F32, tag="bcr", name="bcr")
                u = up.tile([HD, 1024], F32, tag="u", name="u")
                nc.vector.tensor_copy(den, pv[HD:HD + 1, :])
                nc.vector.tensor_copy(u, pv[0:HD, :])
                nc.vector.reciprocal_approx_fast(rec, den)
                nc.gpsimd.partition_broadcast(bcr, rec[0:1, :])
                nc.vector.tensor_mul(
                    at[p0:p0 + 64, pair, q0:q0 + 1024], u, bcr)

            # ---- emission schedule ----
            # NOTE: consumers must be EMITTED after their producers (Tile
            # tracks dependencies in emission order), so projection fillers
            # are placed with enough lead before the k-iters that read them.
            K0 = [proj_group(wk, kt, bks, 0, i) for i in range(4)]
            Q0 = [proj_group(wq, qt, bqs, 0, i) for i in range(4)]
            K1 = [proj_group(wk, kt, bks, 1, i) for i in range(4)]
            Q1 = [proj_group(wq, qt, bqs, 1, i) for i in range(4)]
            V = [vnat_group(i) for i in range(NB)]
            O0 = [oproj_unit(nb, h) for nb in range(8) for h in range(2)]
            O1 = [oproj_unit(nb, h) for nb in range(8, 16) for h in range(2)]

            for g in (K0[0], Q0[0], Q0[1], V[0], V[1], V[2], V[3], V[4], V[5]):
                g()
            attn_head(0, 0, 0, [K0[1], V[6], V[7], K0[2], V[8], V[9],
                                K0[3], V[10], V[11], V[12], V[13], V[14],
                                V[15]])
            attn_head(0, 0, 1, [K1[0], K1[1], K1[2], K1[3],
                                Q1[0], Q1[1], Q1[2], Q1[3]])
            attn_head(1, 0, 0, [Q0[2], Q0[3]])
            attn_head(1, 0, 1, [])
            attn_head(0, 1, 0, O0[0:6])
            attn_head(0, 1, 1, O0[6:12])
            attn_head(1, 1, 0, O0[12:16])
            attn_head(1, 1, 1, [])
            for g in O1:
                g()
    return nc


_CACHE = {}


def _build():
    if "nc" not in _CACHE:
        nc = bacc.Bacc("TRN2", target_bir_lowering=False, debug=False)
        _emit(nc)
        nc.compile()
        _CACHE["nc"] = nc
    return _CACHE["nc"]


def make_in_maps(x, Wq, bq, Wk, bk, Wv, bv, Wo, bo):
    import ml_dtypes
    f32 = np.float32
    bt = ml_dtypes.bfloat16
    dpr = bt if DT_PR == BF16 else f32
    ones_np = np.ones((128, NB, 4), bt if DT_PV == BF16 else f32)
    xTs = [np.ascontiguousarray(np.asarray(x[b], dtype=f32).T).astype(dpr)
           for b in range(B)]
    in_maps = []
    for c in range(8):
        b, r0 = c // 4, (c % 4) * E
        rows = slice(r0, r0 + E)
        in_maps.append({
            "xT": xTs[b],
            "wqT": np.ascontiguousarray(np.asarray(Wq, f32)[rows].T).astype(dpr),
            "wkT": np.ascontiguousarray(np.asarray(Wk, f32)[rows].T).astype(dpr),
            "wvT": np.ascontiguousarray(np.asarray(Wv, f32)[rows].T).astype(dpr),
            "woT": np.ascontiguousarray(
                np.asarray(Wo, f32)[:, rows].T).astype(bt if DT_AT == BF16 else f32),
            "bq2": np.ascontiguousarray(np.asarray(bq, f32)[rows].reshape(2, 128).T),
            "bk2": np.ascontiguousarray(np.asarray(bk, f32)[rows].reshape(2, 128).T),
            "bv1": np.ascontiguousarray(np.asarray(bv, f32)[rows]),
            "vones": ones_np,
        })
    return in_maps


def kernel(x, Wq, bq, Wk, bk, Wv, bv, Wo, bo, _spmd_kwargs=None):
    nc = _build()
    in_maps = make_in_maps(x, Wq, bq, Wk, bk, Wv, bv, Wo, bo)
    res = run_bass_kernel_spmd(nc, in_maps, core_ids=list(range(8)),
                               **(_spmd_kwargs or {}))
    parts = np.stack([res.results[c]["out"] for c in range(8)])
    outv = parts.reshape(B, 4, N, D).sum(axis=1) + np.asarray(bo, np.float32)
    if _spmd_kwargs:
        _CACHE["last_results"] = res
    return outv.astype(np.float32)
